# revision 1
# baseline (speedup 1.0000x reference)
"""BinaryWeightConv2d on Trainium2 — 8-core data-parallel over batch.

Reference computation (fp32):
    scale = clip(mean|w| over (in,kh,kw), 1e-8)          # per out-channel
    bw    = sign(w) * scale
    out   = conv2d(x, bw, stride 1, pad 1) + bias
    y     = ternary(out): 1 if out > 0.5, -1 if out < -0.5, else 0

Kernel strategy:
  - Shard the batch (32) over 8 cores, 4 images each; replicate the tiny
    binarized weights (per the data-parallel sharding hint).
  - Host side: binarize weights to +-1 sign matrices; fold scale & bias into
    per-output-channel thresholds  hi = (0.5-b)/s,  lo = (-0.5-b)/s, so the
    device only computes the +-1 convolution and two compares.
  - Device: conv = 9 shifted-window matmuls (3x3 taps) accumulating in PSUM;
    contraction over C=128 = the partition dim.  x is host-padded to 58x58
    per image so every tap window is one contiguous SBUF slice.
  - Matmul dtype: fp16 hi/lo pair (x = x_h + x_l, both fp16; +-1 weights are
    exact in fp16) -> 18 accumulating matmuls per PSUM tile.  Result matches
    fp32 accumulation to ~1e-7 relative (measured on HW), giving a final
    ternary relative error ~5.7e-4 (the intrinsic fp32 reordering noise
    level).  A float32r variant ("f32r") is ~1.5x faster on 8 cores but has
    ~9e-3 ternary relative error (reduced-precision PE multiplies).
  - Epilogue per PSUM tile, 2 vector ops:  b = (raw < lo);
    y = (raw > hi) - b   in {-1, 0, 1}.
  - Outputs are stored as full padded [C, 464] tiles (contiguous DMA
    segments); the host strips the 2 junk columns per 58-wide row.
"""

import os
import numpy as np

N, C, H, W = 32, 128, 56, 56
O = 256
NCORES = 8
NPC = N // NCORES           # images per core
HP, WP = H + 2, W + 2       # padded spatial
IMG = HP * WP               # 3364
XCOLS = NPC * IMG           # 13456
XCOLS_PAD = XCOLS + 64      # slack: the last tap of the last tile overreads 1
RB = 8                      # output rows per PSUM tile
NT = RB * WP                # 464 = PSUM tile free size (<= 512 bank limit)
NBLK = H // RB              # 7 row blocks
TAPS = [(kh, kw) for kh in range(3) for kw in range(3)]

MODE = os.environ.get("BWC_MODE", "f16p+j+ys")

_prog_cache = {}


def _build(mode, repeat=1):
    import concourse.tile as tile
    from concourse import mybir, bacc
    from contextlib import ExitStack

    dt = mybir.dt
    nc = bacc.Bacc()

    parts = mode.split("+")
    base, flags = parts[0], set(parts[1:])
    jpad = "j" in flags
    ydt_bf = "h" in flags       # store ternary output as bf16 (host converts)
    dmaless = "dl" in flags     # timing probe: ~zero output DMA volume
    act_out = "a" in flags      # issue output stores on the ACT HWDGE ring
    obufs = 16 if "o16" in flags else 6
    ysplit = "ys" in flags      # dedicated deep pool for DMA-held y tiles

    if base == "f16p":
        mm_dt, np_mm = dt.float16, np.float16
    elif base == "f32r":
        mm_dt, np_mm = dt.float32r, np.float32
    elif base == "f32":
        mm_dt, np_mm = dt.float32, np.float32
    else:
        raise ValueError(mode)
    pair = base == "f16p"

    xh_d = nc.declare_dram_parameter("xh", [C, XCOLS_PAD], mm_dt, isOutput=False)
    xl_d = (nc.declare_dram_parameter("xl", [C, XCOLS_PAD], mm_dt, isOutput=False)
            if pair else None)
    sw_d = nc.declare_dram_parameter("sw", [C, 9 * O], mm_dt, isOutput=False)
    thr_d = nc.declare_dram_parameter("thr", [C, 4], dt.float32, isOutput=False)
    out_dt = dt.bfloat16 if ydt_bf else dt.float32
    if jpad:
        out_d = nc.declare_dram_parameter("out", [NPC, 2, NBLK, C, NT],
                                          out_dt, isOutput=True)
    else:
        out_d = nc.declare_dram_parameter("out", [2, C, NPC, H, W],
                                          out_dt, isOutput=True)

    with tile.TileContext(nc) as tc, ExitStack() as ctx:
        inp = ctx.enter_context(tc.tile_pool(name="inp", bufs=2))
        outp = ctx.enter_context(tc.tile_pool(name="outp", bufs=4 if ysplit else obufs))
        ypool = (ctx.enter_context(tc.tile_pool(name="ypool", bufs=24))
                 if ysplit else outp)
        psum = ctx.enter_context(tc.tile_pool(name="psum", bufs=8, space="PSUM"))

        def body():
            t_w = inp.tile([C, 9 * O], mm_dt, tag="w")
            nc.sync.dma_start(t_w[:], sw_d[:])
            t_thr = inp.tile([C, 4], dt.float32, tag="thr")
            nc.sync.dma_start(t_thr[:], thr_d[:])

            t_xh = inp.tile([C, XCOLS_PAD], mm_dt, tag="xh")
            t_xl = (inp.tile([C, XCOLS_PAD], mm_dt, tag="xl", name="t_xl")
                    if pair else None)
            # chunked x loads (per image) so compute starts after chunk 0
            bounds = [0, IMG, 2 * IMG, 3 * IMG, XCOLS_PAD]
            for i in range(4):
                lo, hi = bounds[i], bounds[i + 1]
                nc.sync.dma_start(t_xh[:, lo:hi], xh_d[:, lo:hi])
                if pair:
                    nc.sync.dma_start(t_xl[:, lo:hi], xl_d[:, lo:hi])

            nmm = 18 if pair else 9
            for n in range(NPC):
                for oc in range(2):
                    hi_ap = t_thr[:, 2 * oc:2 * oc + 1]
                    lo_ap = t_thr[:, 2 * oc + 1:2 * oc + 2]
                    for j in range(NBLK):
                        h0 = j * RB
                        pt = psum.tile([C, 512], dt.float32, tag="pt")
                        pt = pt[:, :NT]
                        k = 0
                        for t, (kh, kw) in enumerate(TAPS):
                            base_off = n * IMG + (h0 + kh) * WP + kw
                            wt = t_w[:, t * O + oc * C: t * O + oc * C + C]
                            nc.tensor.matmul(pt, wt, t_xh[:, base_off:base_off + NT],
                                             start=(k == 0), stop=(k == nmm - 1))
                            k += 1
                            if pair:
                                nc.tensor.matmul(pt, wt, t_xl[:, base_off:base_off + NT],
                                                 start=False, stop=(k == nmm - 1))
                                k += 1
                        # ternary epilogue: y = (raw > hi) - (raw < lo)
                        b = outp.tile([C, NT], dt.float32, tag="b")
                        nc.vector.tensor_scalar(b[:], pt, lo_ap, None,
                                                mybir.AluOpType.is_lt)
                        y = ypool.tile([C, NT], out_dt, tag="y")
                        nc.vector.scalar_tensor_tensor(
                            y[:], pt, hi_ap, b[:],
                            mybir.AluOpType.is_gt, mybir.AluOpType.subtract)
                        if jpad:
                            out_eng = nc.scalar if act_out else nc.sync
                            if dmaless:
                                out_eng.dma_start(out_d[n, oc, j][:, :8], y[:, :8])
                            else:
                                out_eng.dma_start(out_d[n, oc, j], y[:])
                        else:
                            y_r = y[:].rearrange("p (r w) -> p r w", w=WP)[:, :, :W]
                            nc.sync.dma_start(out_d[oc, :, n, h0:h0 + RB, :], y_r)

        if repeat == 1:
            body()
        else:
            with tc.For_i(0, repeat, 1):
                body()

    nc.compile()
    return nc, np_mm


def _host_prep(x, weight, bias):
    scale = np.clip(np.mean(np.abs(weight), axis=(1, 2, 3)), 1e-8, None)  # [O]
    sw = np.sign(weight)                                                  # [O,C,3,3]
    hi = ((0.5 - bias.astype(np.float64)) / scale.astype(np.float64)).astype(np.float32)
    lo = ((-0.5 - bias.astype(np.float64)) / scale.astype(np.float64)).astype(np.float32)
    thr = np.stack([hi[:C], lo[:C], hi[C:], lo[C:]], axis=1).astype(np.float32)
    # lhsT layout: sw[c, t*O + o]
    swt = np.ascontiguousarray(sw.transpose(1, 2, 3, 0).reshape(C, 9 * O))
    # pad x to 58x58 and lay out [C, n*3364 + hp*58 + wp]
    xp = np.zeros((N, C, HP, WP), dtype=np.float32)
    xp[:, :, 1:-1, 1:-1] = x
    xp = xp.transpose(1, 0, 2, 3).reshape(C, N * IMG)
    return thr, swt, xp


def _make_in_maps(mode, thr, swt, xp):
    pair = mode.startswith("f16p")
    in_maps = []
    for c in range(NCORES):
        xc = np.zeros((C, XCOLS_PAD), dtype=np.float32)
        xc[:, :XCOLS] = xp[:, c * XCOLS:(c + 1) * XCOLS]
        m = {"thr": thr}
        if pair:
            xh = xc.astype(np.float16)
            m["xh"] = xh
            m["xl"] = (xc - xh.astype(np.float32)).astype(np.float16)
            m["sw"] = swt.astype(np.float16)
        else:
            m["xh"] = xc
            m["sw"] = swt.copy()
        in_maps.append(m)
    return in_maps


def kernel(x, weight, bias):
    from concourse.bass_utils import run_bass_kernel_spmd

    x = np.asarray(x, dtype=np.float32)
    weight = np.asarray(weight, dtype=np.float32)
    bias = np.asarray(bias, dtype=np.float32)

    thr, swt, xp = _host_prep(x, weight, bias)

    mode = MODE
    if mode not in _prog_cache:
        _prog_cache[mode] = _build(mode)
    nc, _ = _prog_cache[mode]

    in_maps = _make_in_maps(mode, thr, swt, xp)
    res = run_bass_kernel_spmd(nc, in_maps, list(range(NCORES)))

    # ---- gather per-core outputs -> [N, O, H, W] fp32 ----
    out = np.empty((N, O, H, W), dtype=np.float32)
    for c in range(NCORES):
        oc_out = res.results[c]["out"]
        if "+j" in mode:
            # [NPC, 2, NBLK, C, NT]: rows of 58, valid w < 56
            v = np.asarray(oc_out).astype(np.float32, copy=False)
            v = v.reshape(NPC, 2, NBLK, C, RB, WP)[:, :, :, :, :, :W]
            v = v.transpose(0, 1, 3, 2, 4, 5).reshape(NPC, O, H, W)
            out[c * NPC:(c + 1) * NPC] = v
        else:
            for oc in range(2):
                out[c * NPC:(c + 1) * NPC, oc * C:(oc + 1) * C] = \
                    oc_out[oc].transpose(1, 0, 2, 3)
    return out



# revision 43
# speedup vs baseline: 1.9972x; 1.9972x over previous
"""BinaryWeightConv2d on Trainium2 — 8-core data-parallel over batch.

Reference computation (fp32):
    scale = clip(mean|w| over (in,kh,kw), 1e-8)          # per out-channel
    bw    = sign(w) * scale
    out   = conv2d(x, bw, stride 1, pad 1) + bias
    y     = ternary(out): 1 if out > 0.5, -1 if out < -0.5, else 0

Kernel strategy (mode f16s+j+ys, ~160us on 8 cores vs 338us f16p baseline):
  - Shard the batch (32) over 8 cores, 4 images each; replicate the tiny
    binarized weights (per the data-parallel sharding hint).
  - Host side: binarize weights to +-1 sign matrices; fold scale & bias into
    per-output-channel thresholds  hi = (0.5-b)/s,  lo = (-0.5-b)/s, so the
    device only computes the +-1 convolution and two compares.
  - Device: conv = 9 shifted-window matmuls (3x3 taps) accumulating in PSUM;
    contraction over C=128 = the partition dim.  x is host-padded to 58x58
    per image so every tap window is one contiguous SBUF slice.
  - Matmul dtype fp16 SINGLE (x rounded to fp16; +-1 weights exact): 9
    matmuls per PSUM tile.  Ternary rel err 1.24e-2 (deterministic, under
    the 2e-2 gate).  The fp16 hi/lo pair mode "f16p" is exact (5.7e-4) but
    2.1x slower.  fp8 DoubleRow modes: "fp8p" (hi/lo e4m3 pair fused into 9
    K=256 DR matmuls) runs ~141us but fails the gate at 2.16e-2; "fp8t+x4"
    (triple plane) passes at 8.05e-3 but lands ~225us — per-instruction
    overhead (LDWEIGHTS+dispatch ~130-230ns/matmul) outweighs the 2x fp8
    stream rate at this tile size (PSUM bank caps moving size at 512 fp32).
  - Epilogue per PSUM tile, 2 vector ops:  b = (raw < lo);
    y = (raw > hi) - b   in {-1, 0, 1}.
  - Outputs are stored as full padded [C, 464] tiles (contiguous DMA
    segments); the host strips the 2 junk columns per 58-wide row.

Measured on HW (pairmed/min of interleaved R-loop differencing):
  f16s+j+ys 158-164us rel 1.24e-2   | fp8p+j+ys 141us rel 2.16e-2 (FAIL)
  fp8t+j+ys+x4 225us rel 8.05e-3    | f16p+j+ys (old baseline) 338us
Dead ends measured: weight-stationary reordering (wg4: +65us — PSUM bank
interleave penalty, no LDW dedup in the IR), multi-bank PSUM tiles
(matmul cannot cross banks), e5m2 third plane (half-rate DR), sg sign
epilogue on ACT (no gain), per-image x tiles / bf16 out / ACT-ring out
(noise-level).
"""
import os
import numpy as np

N, C, H, W = 32, 128, 56, 56
O = 256
NCORES = 8
NPC = N // NCORES           # images per core
HP, WP = H + 2, W + 2       # padded spatial
IMG = HP * WP               # 3364
XCOLS = NPC * IMG           # 13456
XCOLS_PAD = XCOLS + 64      # slack: the last tap of the last tile overreads 1
RB = 8                      # output rows per PSUM tile
NT = RB * WP                # 464 = PSUM tile free size (<= 512 bank limit)
NBLK = H // RB              # 7 row blocks
TAPS = [(kh, kw) for kh in range(3) for kw in range(3)]

MODE = os.environ.get("BWC_MODE", "f16s+j+ys")

_prog_cache = {}


def _build(mode, repeat=1):
    import concourse.tile as tile
    from concourse import mybir, bacc
    from contextlib import ExitStack

    dt = mybir.dt
    nc = bacc.Bacc()

    parts = mode.split("+")
    base, flags = parts[0], set(parts[1:])
    jpad = "j" in flags
    ydt_bf = "h" in flags       # store ternary output as bf16 (host converts)
    dmaless = "dl" in flags     # timing probe: ~zero output DMA volume
    act_out = "a" in flags      # issue output stores on the ACT HWDGE ring
    obufs = 16 if "o16" in flags else 6
    ysplit = "ys" in flags      # dedicated deep pool for DMA-held y tiles
    wg = 0                      # weight-stationary: tap-outer over G psum tiles
    xi = "xi" in flags          # per-image x tiles (cross-iteration overlap)
    noxr = "noxr" in flags      # timing probe: keep xr DMA, skip xr matmuls
    sg = "sg" in flags          # sign-epilogue: 2 ACT Sign + 1 DVE add;
                                # y = sign(pt-hi)+sign(pt-lo) in {-2,0,2},
                                # stored bf16, host halves
    for f in flags:
        if f.startswith("wg"):
            wg = int(f[2:])
    # multi-bank PSUM tiles: fewer, larger matmuls (amortize LDWEIGHTS +
    # per-instruction overhead). nt2: 2 banks/tile, nt4: 4 banks/tile.
    if "nt4" in flags:
        rb, nblk, ntp, pbufs, ybufs = 28, 2, 2048, 2, 8
    elif "nt2" in flags:
        rb, nblk, ntp, pbufs, ybufs = 14, 4, 1024, 4, 12
    else:
        rb, nblk, ntp, pbufs, ybufs = RB, NBLK, 512, 8, 24
    nt = rb * WP

    if base == "f16p":
        mm_dt, np_mm = dt.float16, np.float16
    elif base == "f16s":
        mm_dt, np_mm = dt.float16, np.float16
    elif base in ("fp8p", "fp8t"):
        mm_dt, np_mm = dt.float8e4, None
    elif base == "f32r":
        mm_dt, np_mm = dt.float32r, np.float32
    elif base == "f32":
        mm_dt, np_mm = dt.float32, np.float32
    else:
        raise ValueError(mode)
    pair = base == "f16p"
    dr = base in ("fp8p", "fp8t")  # fp8 planes fused via DoubleRow perf mode
    triple = base == "fp8t"        # extra 3rd-precision plane, tap-paired
    # third plane dtype: e4m3 ("x4", full-rate DR, scales 16/64) or e5m2
    # (denormal-free but half-rate DR on HW)
    x4 = "x4" in flags
    xr_dt = dt.float8e4 if x4 else dt.float8e5

    xh_d = nc.declare_dram_parameter("xh", [C, XCOLS_PAD], mm_dt, isOutput=False)
    xl_d = (nc.declare_dram_parameter("xl", [C, XCOLS_PAD], mm_dt, isOutput=False)
            if (pair or dr) else None)
    xr_d = (nc.declare_dram_parameter("xr", [C, 3, XCOLS_PAD], xr_dt, isOutput=False)
            if triple else None)
    if dr:
        sw_d = nc.declare_dram_parameter("sw", [C, 2, 9 * O], mm_dt, isOutput=False)
    else:
        sw_d = nc.declare_dram_parameter("sw", [C, 9 * O], mm_dt, isOutput=False)
    swl_d = (nc.declare_dram_parameter("swl", [C, 2, 5 * O], xr_dt, isOutput=False)
             if triple else None)
    thr_d = nc.declare_dram_parameter("thr", [C, 4], dt.float32, isOutput=False)
    out_dt = dt.bfloat16 if (ydt_bf or sg) else dt.float32
    if jpad:
        out_d = nc.declare_dram_parameter("out", [NPC, 2, nblk, C, nt],
                                          out_dt, isOutput=True)
    else:
        out_d = nc.declare_dram_parameter("out", [2, C, NPC, H, W],
                                          out_dt, isOutput=True)

    with tile.TileContext(nc) as tc, ExitStack() as ctx:
        inp = ctx.enter_context(tc.tile_pool(name="inp", bufs=1))
        outp = ctx.enter_context(tc.tile_pool(name="outp", bufs=4 if ysplit else obufs))
        ypool = (ctx.enter_context(tc.tile_pool(name="ypool", bufs=ybufs))
                 if ysplit else outp)
        psum = ctx.enter_context(tc.tile_pool(name="psum", bufs=pbufs, space="PSUM"))

        def body():
            if dr:
                t_w = inp.tile([C, 2, 9 * O], mm_dt, tag="w")
                nc.sync.dma_start(t_w[:], sw_d[:])
            else:
                t_w = inp.tile([C, 9 * O], mm_dt, tag="w")
                nc.sync.dma_start(t_w[:], sw_d[:])
            if triple:
                t_wl = inp.tile([C, 2, 5 * O], xr_dt, tag="wl")
                nc.sync.dma_start(t_wl[:], swl_d[:])
            t_thr = inp.tile([C, 4], dt.float32, tag="thr")
            nc.sync.dma_start(t_thr[:], thr_d[:])

            IMG64 = IMG + 64
            if xi:
                assert dr
                t_xs, t_xrs = [], []
                for n in range(NPC):
                    txn = inp.tile([C, 2, IMG64], mm_dt, tag=f"x{n}",
                                   name=f"t_x{n}")
                    lo = n * IMG
                    nc.sync.dma_start(txn[:, 0], xh_d[:, lo:lo + IMG64])
                    nc.sync.dma_start(txn[:, 1], xl_d[:, lo:lo + IMG64])
                    t_xs.append(txn)
                    if triple:
                        txr = inp.tile([C, 3, IMG64], xr_dt, tag=f"xr{n}",
                                       name=f"t_xr{n}")
                        for pl in range(3):
                            nc.sync.dma_start(txr[:, pl],
                                              xr_d[:, pl, lo:lo + IMG64])
                        t_xrs.append(txr)
            elif dr:
                t_x = inp.tile([C, 2, XCOLS_PAD], mm_dt, tag="x")
            else:
                t_xh = inp.tile([C, XCOLS_PAD], mm_dt, tag="xh")
                t_xl = (inp.tile([C, XCOLS_PAD], mm_dt, tag="xl", name="t_xl")
                        if pair else None)
            t_xr = (inp.tile([C, 3, XCOLS_PAD], xr_dt, tag="xr", name="t_xr")
                    if (triple and not xi) else None)
            if not xi:
                # chunked x loads (per image) so compute starts after chunk 0
                bounds = [0, IMG, 2 * IMG, 3 * IMG, XCOLS_PAD]
                for i in range(4):
                    lo, hi = bounds[i], bounds[i + 1]
                    if dr:
                        nc.sync.dma_start(t_x[:, 0, lo:hi], xh_d[:, lo:hi])
                        nc.sync.dma_start(t_x[:, 1, lo:hi], xl_d[:, lo:hi])
                        if triple:
                            for pl in range(3):
                                nc.sync.dma_start(t_xr[:, pl, lo:hi],
                                                  xr_d[:, pl, lo:hi])
                    else:
                        nc.sync.dma_start(t_xh[:, lo:hi], xh_d[:, lo:hi])
                        if pair:
                            nc.sync.dma_start(t_xl[:, lo:hi], xl_d[:, lo:hi])

            nmm = 18 if pair else 9
            n_ops = 14 if (triple and not noxr) else 9
            # xl tap pairs via pre-shifted SBUF planes [xr<<1, xr, xr<<58]:
            # (plane_start, window offset). Plane pair (0,1) at o covers taps
            # o+1 (plane0) and o (plane1); pair (1,2) at o covers o and o+WP.
            # Last entry: tap (2,2) alone via plane0 at o(2,2)-1, zero wt on
            # plane 1.
            XLP = [(0, 0), (0, WP), (0, 2 * WP), (1, 2), (0, 2 * WP + 1)]

            def mm_op(pt, n, oc, j, t, start, stop):
                blk_off = (0 if xi else n * IMG) + j * rb * WP
                if triple and t >= 9:
                    p = t - 9
                    ps, od = XLP[p]
                    o0 = blk_off + od
                    txr = t_xrs[n] if xi else t_xr
                    rhs = txr[:, ps:ps + 2, o0:o0 + nt]
                    wt = t_wl[:, :, p * O + oc * C: p * O + oc * C + C]
                    nc.tensor.matmul(pt, wt, rhs, start=start, stop=stop,
                                     perf_mode=mybir.MatmulPerfMode.DoubleRow)
                    return
                kh, kw = TAPS[t]
                base_off = blk_off + kh * WP + kw
                if dr:
                    wt = t_w[:, :, t * O + oc * C: t * O + oc * C + C]
                    tx = t_xs[n] if xi else t_x
                    nc.tensor.matmul(
                        pt, wt, tx[:, :, base_off:base_off + nt],
                        start=start, stop=stop,
                        perf_mode=mybir.MatmulPerfMode.DoubleRow)
                    return
                wt = t_w[:, t * O + oc * C: t * O + oc * C + C]
                nc.tensor.matmul(pt, wt, t_xh[:, base_off:base_off + nt],
                                 start=start, stop=stop and not pair)
                if pair:
                    nc.tensor.matmul(pt, wt, t_xl[:, base_off:base_off + nt],
                                     start=False, stop=stop)

            def epilogue(pt, n, oc, j):
                hi_ap = t_thr[:, 2 * oc:2 * oc + 1]
                lo_ap = t_thr[:, 2 * oc + 1:2 * oc + 2]
                if sg:
                    # y2 = sign(pt - hi) + sign(pt - lo) in {-2,0,2}
                    # (thr holds NEGATED thresholds in sg mode; ACT computes
                    # Sign(pt*1 + bias))
                    s1 = outp.tile([C, nt], dt.bfloat16, tag="s1")
                    nc.scalar.activation(s1[:], pt,
                                         mybir.ActivationFunctionType.Sign,
                                         bias=hi_ap)
                    s2 = outp.tile([C, nt], dt.bfloat16, tag="s2")
                    nc.scalar.activation(s2[:], pt,
                                         mybir.ActivationFunctionType.Sign,
                                         bias=lo_ap)
                    y = ypool.tile([C, nt], out_dt, tag="y")
                    nc.vector.scalar_tensor_tensor(
                        y[:], s1[:], 0.0, s2[:],
                        mybir.AluOpType.add, mybir.AluOpType.add)
                else:
                    # ternary epilogue: y = (raw > hi) - (raw < lo)
                    b = outp.tile([C, nt], dt.float32, tag="b")
                    nc.vector.tensor_scalar(b[:], pt, lo_ap, None,
                                            mybir.AluOpType.is_lt)
                    y = ypool.tile([C, nt], out_dt, tag="y")
                    nc.vector.scalar_tensor_tensor(
                        y[:], pt, hi_ap, b[:],
                        mybir.AluOpType.is_gt, mybir.AluOpType.subtract)
                if jpad:
                    out_eng = nc.scalar if act_out else nc.sync
                    if dmaless:
                        out_eng.dma_start(out_d[n, oc, j][:, :8], y[:, :8])
                    else:
                        out_eng.dma_start(out_d[n, oc, j], y[:])
                else:
                    y_r = y[:].rearrange("p (r w) -> p r w", w=WP)[:, :, :W]
                    nc.sync.dma_start(out_d[oc, :, n, j * rb:j * rb + rb, :], y_r)

            if wg:
                # tap-outer over groups of `wg` row-blocks: consecutive
                # matmuls share stationary weights (amortizes LDWEIGHTS)
                tiles = [(n, oc, j)
                         for oc in range(2) for n in range(NPC)
                         for j in range(nblk)]
                for g0 in range(0, len(tiles), wg):
                    grp = tiles[g0:g0 + wg]
                    pts = []
                    for _ in grp:
                        pt = psum.tile([C, ntp], dt.float32, tag="pt")
                        pts.append(pt[:, :nt])
                    for t in range(n_ops):
                        for pt, (n, oc, j) in zip(pts, grp):
                            mm_op(pt, n, oc, j, t,
                                  start=(t == 0), stop=(t == n_ops - 1))
                    for pt, (n, oc, j) in zip(pts, grp):
                        epilogue(pt, n, oc, j)
            else:
                for n in range(NPC):
                    for oc in range(2):
                        for j in range(nblk):
                            pt = psum.tile([C, ntp], dt.float32, tag="pt")
                            pt = pt[:, :nt]
                            for t in range(n_ops):
                                mm_op(pt, n, oc, j, t,
                                      start=(t == 0), stop=(t == n_ops - 1))
                            epilogue(pt, n, oc, j)

        if repeat == 1:
            body()
        else:
            with tc.For_i(0, repeat, 1):
                body()

    nc.compile()
    return nc, np_mm


def _host_prep(x, weight, bias):
    scale = np.clip(np.mean(np.abs(weight), axis=(1, 2, 3)), 1e-8, None)  # [O]
    sw = np.sign(weight)                                                  # [O,C,3,3]
    hi = ((0.5 - bias.astype(np.float64)) / scale.astype(np.float64)).astype(np.float32)
    lo = ((-0.5 - bias.astype(np.float64)) / scale.astype(np.float64)).astype(np.float32)
    thr = np.stack([hi[:C], lo[:C], hi[C:], lo[C:]], axis=1).astype(np.float32)
    # lhsT layout: sw[c, t*O + o]
    swt = np.ascontiguousarray(sw.transpose(1, 2, 3, 0).reshape(C, 9 * O))
    # pad x to 58x58 and lay out [C, n*3364 + hp*58 + wp]
    xp = np.zeros((N, C, HP, WP), dtype=np.float32)
    xp[:, :, 1:-1, 1:-1] = x
    xp = xp.transpose(1, 0, 2, 3).reshape(C, N * IMG)
    return thr, swt, xp


def _make_in_maps(mode, thr, swt, xp):
    base = mode.split("+")[0]
    f8 = None
    if base in ("fp8p", "fp8t"):
        import ml_dtypes
        f8 = ml_dtypes.float8_e4m3fn
    in_maps = []
    sg = "sg" in mode.split("+")[1:]
    for c in range(NCORES):
        xc = np.zeros((C, XCOLS_PAD), dtype=np.float32)
        xc[:, :XCOLS] = xp[:, c * XCOLS:(c + 1) * XCOLS]
        # sg epilogue computes Sign(pt + bias): bias = NEGATED thresholds
        m = {"thr": -thr if sg else thr}
        if base == "f16p":
            xh = xc.astype(np.float16)
            m["xh"] = xh
            m["xl"] = (xc - xh.astype(np.float32)).astype(np.float16)
            m["sw"] = swt.astype(np.float16)
        elif base == "f16s":
            m["xh"] = xc.astype(np.float16)
            m["sw"] = swt.astype(np.float16)
        elif base == "fp8p":
            # hi/lo split: x ~= xh + xl/16, both e4m3 (lo scaled x16 to stay
            # in the normal range); weight planes (+-1, +-1/16) fold the scale.
            xh = xc.astype(f8)
            m["xh"] = xh
            m["xl"] = ((xc - xh.astype(np.float32)) * 16.0).astype(f8)
            m["sw"] = np.stack([swt, swt / 16.0], axis=1).astype(f8)
        elif base == "fp8t":
            # triple split: x ~= xh + xm/s2 + xr/s3, planes tap-paired via
            # pre-shifted copies [xr<<1, xr, xr<<WP].
            # x4: all-e4m3, scales (16, 64) — weights +-1/64 stay normal,
            #     xr data dips into subnormals (HW-validated 8.05e-3).
            # else: xr pass in e5m2, scales (64, 4096) — denormal-free but
            #     half-rate DR on HW.
            import ml_dtypes
            if "x4" in mode.split("+")[1:]:
                fxr, s2, s3 = f8, 16.0, 64.0
            else:
                fxr, s2, s3 = ml_dtypes.float8_e5m2, 64.0, 4096.0
            xh = xc.astype(f8)
            xhf = xh.astype(np.float32)
            xm = ((xc - xhf) * s2).astype(f8)
            xr = ((xc - xhf - xm.astype(np.float32) / s2) * s3).astype(fxr)
            m["xh"], m["xl"] = xh, xm
            xr3 = np.zeros((C, 3, XCOLS_PAD), dtype=xr.dtype)
            xr3[:, 0, :-1] = xr[:, 1:]      # xr << 1
            xr3[:, 1] = xr
            xr3[:, 2, :-WP] = xr[:, WP:]    # xr << WP
            m["xr"] = xr3
            m["sw"] = np.stack([swt, swt / s2], axis=1).astype(f8)
            swl = np.zeros((C, 2, 5 * O), dtype=np.float32)
            for p, (t0, t1) in enumerate([(1, 0), (4, 3), (7, 6), (2, 5), (8, None)]):
                swl[:, 0, p * O:(p + 1) * O] = swt[:, t0 * O:(t0 + 1) * O] / s3
                if t1 is not None:
                    swl[:, 1, p * O:(p + 1) * O] = swt[:, t1 * O:(t1 + 1) * O] / s3
            m["swl"] = swl.astype(fxr)
        else:
            m["xh"] = xc
            m["sw"] = swt.copy()
        in_maps.append(m)
    return in_maps


def kernel(x, weight, bias):
    from concourse.bass_utils import run_bass_kernel_spmd

    x = np.asarray(x, dtype=np.float32)
    weight = np.asarray(weight, dtype=np.float32)
    bias = np.asarray(bias, dtype=np.float32)

    thr, swt, xp = _host_prep(x, weight, bias)

    mode = MODE
    if mode not in _prog_cache:
        _prog_cache[mode] = _build(mode)
    nc, _ = _prog_cache[mode]

    in_maps = _make_in_maps(mode, thr, swt, xp)
    res = run_bass_kernel_spmd(nc, in_maps, list(range(NCORES)))
    return _gather(res, mode)


def _gather(res, mode):
    flags = set(mode.split("+")[1:])
    if "nt4" in flags:
        rb, nblk = 28, 2
    elif "nt2" in flags:
        rb, nblk = 14, 4
    else:
        rb, nblk = RB, NBLK
    # ---- gather per-core outputs -> [N, O, H, W] fp32 ----
    out = np.empty((N, O, H, W), dtype=np.float32)
    for c in range(NCORES):
        oc_out = res.results[c]["out"]
        if "+j" in mode:
            # [NPC, 2, nblk, C, rb*WP]: rows of 58, valid w < 56
            v = np.asarray(oc_out).astype(np.float32, copy=False)
            if "sg" in flags:
                v = v * np.float32(0.5)   # sg stores {-2,0,2}
            v = v.reshape(NPC, 2, nblk, C, rb, WP)[:, :, :, :, :, :W]
            v = v.transpose(0, 1, 3, 2, 4, 5).reshape(NPC, O, H, W)
            out[c * NPC:(c + 1) * NPC] = v
        else:
            for oc in range(2):
                out[c * NPC:(c + 1) * NPC, oc * C:(oc + 1) * C] = \
                    oc_out[oc].transpose(1, 0, 2, 3)
    return out

